# revision 66
# baseline (speedup 1.0000x reference)
"""AgentImputer Trainium2 kernel.

Contract: kernel(**inputs) takes the FULL unsharded inputs (as produced by
reference.setup_inputs()) and returns the FULL output [64, 40, 2] float32.

Strategy: data-parallel over batch B=64 across 8 NeuronCores (8 batches /
core -> 320 folded LSTM rows per core). Tiny LSTM/GCN weights are
replicated. The 128-step TimeLSTM scan runs feature-major ([hid, row]
tiles) so every matmul contracts along partitions; categorical embeddings
are folded into the input matmul via one-hot rows; biases are folded into
the matmuls via a constant-1 row appended to the h/c state; the per-graph
GCN (shared edge_index) becomes dense [40,40] mean-aggregation matmuls.
Matmul operands use float32r (single-pass fp32 streaming when N>=256).
"""

import sys

import numpy as np

sys.path.insert(0, "/opt/trn_rl_repo")

# ---------------------------------------------------------------- constants
B, W, N, F_IN = 64, 128, 40, 66
HID = 100
NUM_CONT = 64
NCLS_POS, NCLS_TEAM = 16, 9
EMB_POS, EMB_TEAM = 4, 3
NCORES = 8
BL = B // NCORES          # 8 local batch elems per core
R = BL * N                # 320 rows per core; row j = 40*b_local + n
OH_P0 = 66                # one-hot pos cols [66:83)
OH_T0 = 83                # one-hot team cols [83:100) (entries 10..16 pad)
XC = 100                  # xs tile feature columns
G4 = 4 * HID


# ---------------------------------------------------------------- host prep
def _host_weights(inputs):
    f32 = np.float32
    Uall_w = np.asarray(inputs["Uall_w"], f32)       # [400, 71]
    Uall_b = np.asarray(inputs["Uall_b"], f32)       # [400]
    Wall_w = np.asarray(inputs["Wall_w"], f32)       # [400, 100]
    Wall_b = np.asarray(inputs["Wall_b"], f32)       # [400]
    Wd_w = np.asarray(inputs["Wd_w"], f32)           # [100, 100]
    Wd_b = np.asarray(inputs["Wd_b"], f32)           # [100]
    lin_w = np.asarray(inputs["lin_w"], f32)         # [100, 100]
    lin_b = np.asarray(inputs["lin_b"], f32)         # [100]
    emb_pos = np.asarray(inputs["emb_pos"], f32)     # [16, 4]
    emb_team = np.asarray(inputs["emb_team"], f32)   # [9, 3]
    edge_index = np.asarray(inputs["edge_index"]).astype(np.int64)  # [2, E]

    # Input-side weights [100, 400]: rows 0:64 continuous features; rows
    # 64,65 (raw categorical codes riding along in the transposed tile) get
    # zero weights; rows 66:83 / 83:93 are one-hot rows with the embedding
    # tables pre-multiplied in (code 0 == missing -> zero row); 93:100 pad.
    WxT = np.zeros((XC, G4), f32)
    WxT[0:NUM_CONT] = Uall_w[:, 0:NUM_CONT].T
    pad_pos = np.vstack([np.zeros((1, EMB_POS), f32), emb_pos])    # [17, 4]
    pad_team = np.vstack([np.zeros((1, EMB_TEAM), f32), emb_team])  # [10, 3]
    WxT[OH_P0:OH_T0] = pad_pos @ Uall_w[:, NUM_CONT:NUM_CONT + EMB_POS].T
    WxT[OH_T0:OH_T0 + NCLS_TEAM + 1] = (
        pad_team @ Uall_w[:, NUM_CONT + EMB_POS:].T
    )

    # h-side weights with the full gate bias folded in as an extra row
    # (state tiles carry a constant-1 row at partition HID).
    WallT = np.concatenate([Wall_w.T, (Wall_b + Uall_b)[None, :]], 0)  # [101, 400]
    WdT = np.concatenate([Wd_w.T, Wd_b[None, :]], 0)                   # [101, 100]
    linT = np.concatenate([lin_w.T, lin_b[None, :]], 0)                # [101, 100]

    # Mean-aggregation matrix: M[s, d] = count(s->d) / max(deg(d), 1)
    src, dst = edge_index[0], edge_index[1]
    cnt = np.zeros((N, N), f32)
    np.add.at(cnt, (src, dst), 1.0)
    deg = np.maximum(cnt.sum(axis=0), 1.0)
    Mmat = cnt / deg[None, :]

    # iota rows for the merged one-hot compare: [0..16 | 0..9, -1 x7],
    # replicated for each timestep of an 8-step block
    iota2 = np.concatenate([
        np.arange(NCLS_POS + 1, dtype=f32),
        np.concatenate([np.arange(NCLS_TEAM + 1, dtype=f32),
                        -np.ones(17 - (NCLS_TEAM + 1), f32)]),
    ])
    iota2b = np.tile(iota2, 8)  # [8*34]

    import ml_dtypes
    bf = ml_dtypes.bfloat16
    return {
        "WxT": WxT.astype(bf),
        "WallT": WallT,
        "WdT": WdT,
        "linT": linT,
        "Mmat": np.ascontiguousarray(Mmat, f32),
        "s1l": np.ascontiguousarray(np.asarray(inputs["sage1_l"], f32).T),   # [100, 64]
        "s1r": np.ascontiguousarray(np.asarray(inputs["sage1_r"], f32).T),   # [100, 64]
        "s1b": np.ascontiguousarray(np.asarray(inputs["sage1_lb"], f32)[:, None]),  # [64, 1]
        "s2l": np.ascontiguousarray(np.asarray(inputs["sage2_l"], f32).T),   # [64, 32]
        "s2r": np.ascontiguousarray(np.asarray(inputs["sage2_r"], f32).T),   # [64, 32]
        "s2b": np.ascontiguousarray(np.asarray(inputs["sage2_lb"], f32)[:, None]),  # [32, 1]
        "ow": np.ascontiguousarray(np.asarray(inputs["out_w"], f32).T),      # [32, 2]
        "ob": np.ascontiguousarray(np.asarray(inputs["out_b"], f32)[:, None]),      # [2, 1]
        "iota2b": np.tile(iota2b, (120, 1)).astype(bf),                                # [120, 272]
        "hcinit": np.concatenate(
            [np.zeros((HID, R), f32), np.ones((1, R), f32)], 0
        ),  # [101, R]: zero state + constant-1 bias row
        "ident": np.eye(128, dtype=f32),
        "identb": np.eye(128, dtype=bf),
    }


# ---------------------------------------------------------------- device IR
def build_module(Wsteps=W):
    import concourse.bass as bass
    import concourse.tile as tile
    from concourse import bacc, mybir

    f32 = mybir.dt.float32
    f32r = mybir.dt.float32r
    bf16 = mybir.dt.bfloat16
    AF = mybir.ActivationFunctionType
    EQ = mybir.AluOpType.is_equal
    PSUM = bass.MemorySpace.PSUM

    def r(ap):
        # float32r view: same 4-byte data, single-pass matmul when N>=256
        return ap.bitcast(f32r)

    nc = bacc.Bacc(
        "TRN2", target_bir_lowering=False, debug=False, num_devices=NCORES
    )

    # All matmul-feeding tensors are float32r end-to-end (host arrays stay
    # np.float32; f32r is the same 4-byte encoding).
    X_in = nc.declare_dram_parameter("X", [BL, W, N, F_IN], bf16, isOutput=False)
    ts_in = nc.declare_dram_parameter("ts", [BL, W, N], f32r, isOutput=False)
    w_in = {}
    bf16_params = {"WxT", "iota2b", "identb"}
    for name, shape in [
        ("WxT", [XC, G4]), ("WallT", [HID + 1, G4]), ("WdT", [HID + 1, HID]),
        ("linT", [HID + 1, HID]), ("Mmat", [N, N]),
        ("s1l", [HID, 64]), ("s1r", [HID, 64]), ("s1b", [64, 1]),
        ("s2l", [64, 32]), ("s2r", [64, 32]), ("s2b", [32, 1]),
        ("ow", [32, 2]), ("ob", [2, 1]),
        ("iota2b", [120, 8 * 34]), ("hcinit", [HID + 1, R]),
        ("ident", [128, 128]), ("identb", [128, 128]),
    ]:
        w_in[name] = nc.declare_dram_parameter(
            name, shape, bf16 if name in bf16_params else f32r, isOutput=False
        )
    # device-natural layout [k, b, n]; host transposes to [b, n, k]
    out_ext = nc.declare_dram_parameter("out", [2, BL, N], f32, isOutput=True)

    with tile.TileContext(nc) as tc:
        with (
            tc.tile_pool(name="consts", bufs=1) as consts,
            tc.tile_pool(name="state", bufs=1) as state,
        ):
            # ---- load constants / weights
            wt = {}
            for name, ext in w_in.items():
                wt[name] = consts.tile(
                    list(ext.shape), ext.dtype, tag=name, name=name
                )
                nc.gpsimd.dma_start(out=wt[name][:], in_=ext[:])

            # ---- persistent state: h/c feature-major with const-1 bias row
            # (row HID stays 1.0 forever; per-step writes touch rows 0:HID)
            hT = state.tile([HID + 1, R], f32r, tag="hT")
            cT = state.tile([HID + 1, R], f32r, tag="cT")
            nc.gpsimd.dma_start(out=hT[:], in_=w_in["hcinit"][:])
            nc.gpsimd.dma_start(out=cT[:], in_=w_in["hcinit"][:])

            # ---- ts - 1, stored [64, 2, R]: row t at (partition t%64, block t//64)
            tsm1 = state.tile([64, 2, R], f32r, tag="tsm1")
            ts_jp = ts_in.rearrange("b (j p) n -> j p b n", p=64)
            for j in range(2):
                nc.sync.dma_start(
                    out=tsm1[:, j, :].rearrange("p (b n) -> p b n", n=N),
                    in_=ts_jp[j],
                )
            nc.vector.tensor_scalar_add(tsm1[:], tsm1[:], -1.0)
            # stage ts-1 to DRAM so per-step partition-broadcast DMAs can
            # read it with a flat 0-step AP (SBUF sources cannot broadcast)
            tsm1_d = nc.dram_tensor("tsm1_d", [64, 2, R], f32)
            nc.sync.dma_start(out=tsm1_d[:], in_=tsm1[:].bitcast(f32))

            nodesT = state.tile([HID, R], f32r, tag="nodesT")

            TB = 8  # timestep block for X/ts prefetch
            Xnb = X_in.rearrange("b t n f -> b n t f")

            with (
                tc.tile_pool(name="xs", bufs=2) as xs_pool,
                tc.tile_pool(name="xf", bufs=4) as xf_pool,
                tc.tile_pool(name="gsb", bufs=3) as gsb_pool,
                tc.tile_pool(name="tsb", bufs=3) as tsb_pool,
                tc.tile_pool(name="work", bufs=3) as work,
                tc.tile_pool(name="pg", bufs=1, space=PSUM) as pg_pool,
                tc.tile_pool(name="pxf", bufs=1, space=PSUM) as pxf_pool,
                tc.tile_pool(name="pd", bufs=1, space=PSUM) as pd_pool,
            ):
                xraw = [None] * 3
                TRIPLES = [(0, 3), (3, 3), (6, 2)]
                for t in range(Wsteps):
                    tl = t % TB
                    if tl == 0:
                        # per-b DMAs stack 3 graphs per tile: [120, TB, 100]
                        for k, (b0, nb) in enumerate(TRIPLES):
                            rows = N * nb
                            xt = xs_pool.tile([120, TB, XC], bf16,
                                              tag=f"xs{k}", name=f"xs{k}")
                            for i in range(nb):
                                nc.sync.dma_start(
                                    out=xt[N * i:N * (i + 1), :, 0:F_IN],
                                    in_=Xnb[b0 + i, :, t:t + TB, :],
                                )
                            # merged one-hot: both categorical cols, all TB
                            # steps, all stacked graphs in one op
                            nc.vector.tensor_tensor(
                                out=xt[:rows, :, OH_P0:XC].rearrange(
                                    "p t (g k) -> p t g k", k=17
                                ),
                                in0=wt["iota2b"][0:rows, :].rearrange(
                                    "p (t g k) -> p t g k", t=TB, k=17
                                ),
                                in1=xt[
                                    :rows, :, NUM_CONT:NUM_CONT + 2
                                ].to_broadcast([rows, TB, 2, 17]),
                                op=EQ,
                            )
                            xraw[k] = xt

                    # ------- per-step transposes -> xfT [100, 320]
                    pxf = pxf_pool.tile([XC, R], bf16, tag="pxf")
                    for k, (b0, nb) in enumerate(TRIPLES):
                        rows = N * nb
                        nc.tensor.transpose(
                            pxf[:, 120 * k:120 * k + rows],
                            xraw[k][:rows, tl, :],
                            wt["identb"][:rows, :rows],
                        )
                    xfT = xf_pool.tile([XC, R], bf16, tag="xfT")
                    nc.any.tensor_copy(out=xfT[:], in_=pxf[:])

                    # ------- ts-1 broadcast across partitions via SWDGE DMA
                    # (gpsimd is otherwise idle; src re-reads one partition)
                    tsb = tsb_pool.tile([HID, R], f32, tag="tsb")
                    ts_row = tsm1_d[t % 64, t // 64, :]
                    nc.gpsimd.dma_start(
                        out=tsb[:],
                        in_=bass.AP(
                            tensor=ts_row.tensor,
                            offset=ts_row.offset,
                            ap=[[0, HID], [1, R]],
                        ),
                    )

                    # ------- c path: c_adj = c + tanh(Wd@c + bd) * (ts-1)
                    pd = pd_pool.tile([HID, R], f32, tag="pd")
                    nc.tensor.matmul(pd, wt["WdT"][:], cT[:], start=True, stop=True)
                    cs1 = work.tile([HID, R], f32, tag="cs1")
                    nc.scalar.activation(cs1[:], pd, AF.Tanh)
                    t1 = work.tile([HID, R], f32, tag="t1")
                    nc.vector.tensor_mul(t1[:], cs1[:], tsb[:])
                    cadj = work.tile([HID, R], f32, tag="cadj")
                    nc.vector.tensor_add(cadj[:], cT[0:HID, :].bitcast(f32), t1[:])

                    # ------- gates: psum[g] = WxT_g.T @ xfT + WallT_g.T @ h1
                    # split across two psum tiles (pgA double-buffered) so
                    # next step's x-side matmuls can start before sigmoid
                    # consumes the previous gates
                    pgA = pg_pool.tile([HID, 2, 512], f32, tag="pgA", bufs=2)
                    pgB = pg_pool.tile([HID, 2, 512], f32, tag="pgB", bufs=1)
                    halves = (pgA, pgB)
                    # order: (f,i) x then h parts first so sigmoid A can
                    # start while (o,ct) matmuls still run
                    for g in (0, 1):
                        nc.tensor.matmul(
                            halves[0][:, g, 0:R],
                            wt["WxT"][:, HID * g:HID * (g + 1)],
                            xfT[:], start=True, stop=False,
                        )
                    for g in (0, 1):
                        nc.tensor.matmul(
                            halves[0][:, g, 0:R],
                            wt["WallT"][:, HID * g:HID * (g + 1)],
                            hT[:], start=False, stop=True,
                        )
                    for g in (2, 3):
                        nc.tensor.matmul(
                            halves[1][:, g - 2, 0:R],
                            wt["WxT"][:, HID * g:HID * (g + 1)],
                            xfT[:], start=True, stop=False,
                        )
                    for g in (2, 3):
                        nc.tensor.matmul(
                            halves[1][:, g - 2, 0:R],
                            wt["WallT"][:, HID * g:HID * (g + 1)],
                            hT[:], start=False, stop=True,
                        )
                    gs = gsb_pool.tile([HID, 4, R], f32, tag="gs")
                    nc.scalar.activation(gs[:, 0:2, :], pgA[:, :, 0:R], AF.Sigmoid)
                    nc.scalar.activation(gs[:, 2:4, :], pgB[:, :, 0:R], AF.Sigmoid)

                    # ------- state update: c = f*c_adj + i*ct ; h = o*tanh(c)
                    t2 = work.tile([HID, R], f32, tag="t2")
                    nc.vector.tensor_mul(t2[:], gs[:, 0, :], cadj[:])
                    t3 = work.tile([HID, R], f32, tag="t3")
                    nc.vector.tensor_mul(t3[:], gs[:, 1, :], gs[:, 3, :])
                    nc.vector.tensor_add(cT[0:HID, :], t2[:], t3[:])
                    tnc = work.tile([HID, R], f32, tag="tnc")
                    nc.scalar.activation(tnc[:], cT[0:HID, :].bitcast(f32), AF.Tanh)
                    nc.vector.tensor_mul(hT[0:HID, :], gs[:, 2, :], tnc[:])

                # ---- output linear: nodes = relu(lin @ h + lb)
                pl = pd_pool.tile([HID, R], f32, tag="pd")
                nc.tensor.matmul(pl, wt["linT"][:], hT[:], start=True, stop=True)
                nc.scalar.activation(nodesT[:], pl, AF.Relu)

            # ---------------- GCN: two SAGE layers + output proj
            with (
                tc.tile_pool(name="gc", bufs=2) as gc,
                tc.tile_pool(name="gcs", bufs=1) as gcs,
                tc.tile_pool(name="gp", bufs=2, space=PSUM) as gp,
                tc.tile_pool(name="gp1", bufs=1, space=PSUM) as gp1,
            ):
                def mean_agg(srcT, hid):
                    """srcT: [hid, R] feature-major -> aggT [hid, R]."""
                    aggT = gcs.tile([hid, R], f32r, tag=f"agg{hid}", name="aggT")
                    for b in range(BL):
                        cols = srcT[:, N * b:N * (b + 1)]   # [hid, 40] graph b
                        ptr = gp.tile([N, 128], f32, tag="ptr")
                        nc.tensor.transpose(
                            r(ptr[:, 0:hid]), cols, wt["ident"][:hid, :hid]
                        )
                        nbm = gc.tile([N, 128], f32r, tag="nbm")
                        nc.any.tensor_copy(out=nbm[:, 0:hid], in_=ptr[:, 0:hid])
                        pa = gp.tile([128, N], f32, tag="pa")
                        nc.tensor.matmul(
                            pa[0:hid, :], nbm[:, 0:hid], wt["Mmat"][:],
                            start=True, stop=True,
                        )
                        nc.any.tensor_copy(
                            out=aggT[:, N * b:N * (b + 1)], in_=pa[0:hid, :]
                        )
                    return aggT

                agg1 = mean_agg(nodesT, HID)
                pg1 = gp1.tile([64, R], f32, tag="pg1")
                nc.tensor.matmul(pg1, wt["s1l"][:], agg1[:], start=True, stop=False)
                nc.tensor.matmul(pg1, wt["s1r"][:], nodesT[:], start=False, stop=True)
                g1T = gcs.tile([64, R], f32r, tag="g1T")
                nc.scalar.activation(g1T[:], pg1, AF.Relu, bias=wt["s1b"][:].bitcast(f32))

                agg2 = mean_agg(g1T, 64)
                pg2 = gp1.tile([32, R], f32, tag="pg2")
                nc.tensor.matmul(pg2, wt["s2l"][:], agg2[:], start=True, stop=False)
                nc.tensor.matmul(pg2, wt["s2r"][:], g1T[:], start=False, stop=True)
                g2T = gcs.tile([32, R], f32r, tag="g2T")
                nc.scalar.activation(g2T[:], pg2, AF.Relu, bias=wt["s2b"][:].bitcast(f32))

                po = gp1.tile([2, R], f32, tag="po")
                nc.tensor.matmul(po, wt["ow"][:], g2T[:], start=True, stop=True)
                oT = gcs.tile([2, R], f32, tag="oT")
                nc.scalar.activation(oT[:], po, AF.Relu, bias=wt["ob"][:].bitcast(f32))

                nc.sync.dma_start(
                    out=out_ext.rearrange("k b n -> k (b n)"), in_=oT[:]
                )

    nc.compile()
    return nc


# ---------------------------------------------------------------- execution
_CACHE = {}


def _get_module():
    if "nc" not in _CACHE:
        _CACHE["nc"] = build_module()
    return _CACHE["nc"]


def make_in_maps(inputs):
    f32 = np.float32
    import ml_dtypes
    X = np.ascontiguousarray(np.asarray(inputs["X"], f32).astype(ml_dtypes.bfloat16))
    ts = np.ascontiguousarray(np.asarray(inputs["ts_list"], f32))
    wts = _host_weights(inputs)
    in_maps = []
    for c in range(NCORES):
        m = {"X": X[c * BL:(c + 1) * BL], "ts": ts[c * BL:(c + 1) * BL]}
        m.update(wts)
        in_maps.append(m)
    return in_maps


def kernel(**inputs) -> np.ndarray:
    from concourse.bass_utils import run_bass_kernel_spmd

    nc = _get_module()
    in_maps = make_in_maps(inputs)
    res = run_bass_kernel_spmd(nc, in_maps, list(range(NCORES)))
    outs = [
        np.transpose(res.results[c]["out"], (1, 2, 0)) for c in range(NCORES)
    ]
    return np.ascontiguousarray(np.concatenate(outs, axis=0).astype(np.float32))


# revision 67
# speedup vs baseline: 1.1630x; 1.1630x over previous
"""AgentImputer Trainium2 kernel.

Contract: kernel(**inputs) takes the FULL unsharded inputs (as produced by
reference.setup_inputs()) and returns the FULL output [64, 40, 2] float32.

Strategy: data-parallel over batch B=64 across 8 NeuronCores (8 batches /
core -> 320 folded LSTM rows per core). Tiny LSTM/GCN weights are
replicated. The 128-step TimeLSTM scan runs feature-major ([hid, row]
tiles) so every matmul contracts along partitions; categorical embeddings
are folded into the input matmul via one-hot rows; biases are folded into
the matmuls via a constant-1 row appended to the h/c state; the per-graph
GCN (shared edge_index) becomes dense [40,40] mean-aggregation matmuls.
Matmul operands use float32r (single-pass fp32 streaming when N>=256).
"""

import sys

import numpy as np

sys.path.insert(0, "/opt/trn_rl_repo")

# ---------------------------------------------------------------- constants
B, W, N, F_IN = 64, 128, 40, 66
HID = 100
NUM_CONT = 64
NCLS_POS, NCLS_TEAM = 16, 9
EMB_POS, EMB_TEAM = 4, 3
NCORES = 8
BL = B // NCORES          # 8 local batch elems per core
R = BL * N                # 320 rows per core; row j = 40*b_local + n
OH_P0 = 66                # one-hot pos cols [66:83)
OH_T0 = 83                # one-hot team cols [83:100) (entries 10..16 pad)
XC = 100                  # xs tile feature columns
G4 = 4 * HID


# ---------------------------------------------------------------- host prep
def _host_weights(inputs):
    f32 = np.float32
    Uall_w = np.asarray(inputs["Uall_w"], f32)       # [400, 71]
    Uall_b = np.asarray(inputs["Uall_b"], f32)       # [400]
    Wall_w = np.asarray(inputs["Wall_w"], f32)       # [400, 100]
    Wall_b = np.asarray(inputs["Wall_b"], f32)       # [400]
    Wd_w = np.asarray(inputs["Wd_w"], f32)           # [100, 100]
    Wd_b = np.asarray(inputs["Wd_b"], f32)           # [100]
    lin_w = np.asarray(inputs["lin_w"], f32)         # [100, 100]
    lin_b = np.asarray(inputs["lin_b"], f32)         # [100]
    emb_pos = np.asarray(inputs["emb_pos"], f32)     # [16, 4]
    emb_team = np.asarray(inputs["emb_team"], f32)   # [9, 3]
    edge_index = np.asarray(inputs["edge_index"]).astype(np.int64)  # [2, E]

    # Input-side weights [100, 400]: rows 0:64 continuous features; rows
    # 64,65 (raw categorical codes riding along in the transposed tile) get
    # zero weights; rows 66:83 / 83:93 are one-hot rows with the embedding
    # tables pre-multiplied in (code 0 == missing -> zero row); 93:100 pad.
    WxT = np.zeros((XC, G4), f32)
    WxT[0:NUM_CONT] = Uall_w[:, 0:NUM_CONT].T
    pad_pos = np.vstack([np.zeros((1, EMB_POS), f32), emb_pos])    # [17, 4]
    pad_team = np.vstack([np.zeros((1, EMB_TEAM), f32), emb_team])  # [10, 3]
    WxT[OH_P0:OH_T0] = pad_pos @ Uall_w[:, NUM_CONT:NUM_CONT + EMB_POS].T
    WxT[OH_T0:OH_T0 + NCLS_TEAM + 1] = (
        pad_team @ Uall_w[:, NUM_CONT + EMB_POS:].T
    )

    # h-side weights with the full gate bias folded in as an extra row
    # (state tiles carry a constant-1 row at partition HID).
    WallT = np.concatenate([Wall_w.T, (Wall_b + Uall_b)[None, :]], 0)  # [101, 400]
    WdT = np.concatenate([Wd_w.T, Wd_b[None, :]], 0)                   # [101, 100]
    linT = np.concatenate([lin_w.T, lin_b[None, :]], 0)                # [101, 100]

    # Mean-aggregation matrix: M[s, d] = count(s->d) / max(deg(d), 1)
    src, dst = edge_index[0], edge_index[1]
    cnt = np.zeros((N, N), f32)
    np.add.at(cnt, (src, dst), 1.0)
    deg = np.maximum(cnt.sum(axis=0), 1.0)
    Mmat = cnt / deg[None, :]

    # iota rows for the merged one-hot compare: [0..16 | 0..9, -1 x7],
    # replicated for each timestep of an 8-step block
    iota2 = np.concatenate([
        np.arange(NCLS_POS + 1, dtype=f32),
        np.concatenate([np.arange(NCLS_TEAM + 1, dtype=f32),
                        -np.ones(17 - (NCLS_TEAM + 1), f32)]),
    ])
    iota2b = np.tile(iota2, 8)  # [8*34]

    import ml_dtypes
    bf = ml_dtypes.bfloat16
    return {
        "WxT": WxT.astype(bf),
        "WallT": WallT,
        "WdT": WdT,
        "linT": linT,
        "Mmat": np.ascontiguousarray(Mmat, f32),
        "s1l": np.ascontiguousarray(np.asarray(inputs["sage1_l"], f32).T),   # [100, 64]
        "s1r": np.ascontiguousarray(np.asarray(inputs["sage1_r"], f32).T),   # [100, 64]
        "s1b": np.ascontiguousarray(np.asarray(inputs["sage1_lb"], f32)[:, None]),  # [64, 1]
        "s2l": np.ascontiguousarray(np.asarray(inputs["sage2_l"], f32).T),   # [64, 32]
        "s2r": np.ascontiguousarray(np.asarray(inputs["sage2_r"], f32).T),   # [64, 32]
        "s2b": np.ascontiguousarray(np.asarray(inputs["sage2_lb"], f32)[:, None]),  # [32, 1]
        "ow": np.ascontiguousarray(np.asarray(inputs["out_w"], f32).T),      # [32, 2]
        "ob": np.ascontiguousarray(np.asarray(inputs["out_b"], f32)[:, None]),      # [2, 1]
        "iota2b": np.tile(iota2b, (120, 1)).astype(bf),                                # [120, 272]
        "hcinit": np.concatenate(
            [np.zeros((HID, R), f32), np.ones((1, R), f32)], 0
        ),  # [101, R]: zero state + constant-1 bias row
        "ident": np.eye(128, dtype=f32),
        "identb": np.eye(128, dtype=bf),
    }


# ---------------------------------------------------------------- device IR
def build_module(Wsteps=W):
    import concourse.bass as bass
    import concourse.tile as tile
    from concourse import bacc, mybir

    f32 = mybir.dt.float32
    f32r = mybir.dt.float32r
    bf16 = mybir.dt.bfloat16
    AF = mybir.ActivationFunctionType
    EQ = mybir.AluOpType.is_equal
    PSUM = bass.MemorySpace.PSUM

    def r(ap):
        # float32r view: same 4-byte data, single-pass matmul when N>=256
        return ap.bitcast(f32r)

    nc = bacc.Bacc(
        "TRN2", target_bir_lowering=False, debug=False, num_devices=NCORES
    )

    # All matmul-feeding tensors are float32r end-to-end (host arrays stay
    # np.float32; f32r is the same 4-byte encoding).
    X_in = nc.declare_dram_parameter("X", [BL, W, N, F_IN], bf16, isOutput=False)
    ts_in = nc.declare_dram_parameter("ts", [BL, W, N], f32r, isOutput=False)
    w_in = {}
    bf16_params = {"WxT", "iota2b", "identb"}
    for name, shape in [
        ("WxT", [XC, G4]), ("WallT", [HID + 1, G4]), ("WdT", [HID + 1, HID]),
        ("linT", [HID + 1, HID]), ("Mmat", [N, N]),
        ("s1l", [HID, 64]), ("s1r", [HID, 64]), ("s1b", [64, 1]),
        ("s2l", [64, 32]), ("s2r", [64, 32]), ("s2b", [32, 1]),
        ("ow", [32, 2]), ("ob", [2, 1]),
        ("iota2b", [120, 8 * 34]), ("hcinit", [HID + 1, R]),
        ("ident", [128, 128]), ("identb", [128, 128]),
    ]:
        w_in[name] = nc.declare_dram_parameter(
            name, shape, bf16 if name in bf16_params else f32r, isOutput=False
        )
    # device-natural layout [k, b, n]; host transposes to [b, n, k]
    out_ext = nc.declare_dram_parameter("out", [2, BL, N], f32, isOutput=True)

    with tile.TileContext(nc) as tc:
        with (
            tc.tile_pool(name="consts", bufs=1) as consts,
            tc.tile_pool(name="state", bufs=1) as state,
        ):
            # ---- load constants / weights
            wt = {}
            for name, ext in w_in.items():
                wt[name] = consts.tile(
                    list(ext.shape), ext.dtype, tag=name, name=name
                )
                nc.gpsimd.dma_start(out=wt[name][:], in_=ext[:])

            # ---- persistent state: h/c feature-major with const-1 bias row
            # (row HID stays 1.0 forever; per-step writes touch rows 0:HID)
            hT = state.tile([HID + 1, R], f32r, tag="hT")
            cT = state.tile([HID + 1, R], f32r, tag="cT")
            nc.gpsimd.dma_start(out=hT[:], in_=w_in["hcinit"][:])
            nc.gpsimd.dma_start(out=cT[:], in_=w_in["hcinit"][:])

            # ---- ts - 1, stored [64, 2, R]: row t at (partition t%64, block t//64)
            tsm1 = state.tile([64, 2, R], f32r, tag="tsm1")
            ts_jp = ts_in.rearrange("b (j p) n -> j p b n", p=64)
            for j in range(2):
                nc.sync.dma_start(
                    out=tsm1[:, j, :].rearrange("p (b n) -> p b n", n=N),
                    in_=ts_jp[j],
                )
            nc.vector.tensor_scalar_add(tsm1[:], tsm1[:], -1.0)
            # stage ts-1 to DRAM so per-step partition-broadcast DMAs can
            # read it with a flat 0-step AP (SBUF sources cannot broadcast)
            tsm1_d = nc.dram_tensor("tsm1_d", [64, 2, R], f32)
            nc.sync.dma_start(out=tsm1_d[:], in_=tsm1[:].bitcast(f32))

            nodesT = state.tile([HID, R], f32r, tag="nodesT")

            TB = 8  # timestep block for X/ts prefetch
            Xnb = X_in.rearrange("b t n f -> b n t f")

            with (
                tc.tile_pool(name="xs", bufs=2) as xs_pool,
                tc.tile_pool(name="xf", bufs=4) as xf_pool,
                tc.tile_pool(name="gsb", bufs=3) as gsb_pool,
                tc.tile_pool(name="tsb", bufs=3) as tsb_pool,
                tc.tile_pool(name="work", bufs=3) as work,
                tc.tile_pool(name="pg", bufs=1, space=PSUM) as pg_pool,
                tc.tile_pool(name="pxf", bufs=1, space=PSUM) as pxf_pool,
                tc.tile_pool(name="pd", bufs=1, space=PSUM) as pd_pool,
            ):
                xraw = [None] * 3
                TRIPLES = [(0, 3), (3, 3), (6, 2)]
                for t in range(Wsteps):
                    tl = t % TB
                    if tl == 0:
                        # per-b DMAs stack 3 graphs per tile: [120, TB, 100]
                        for k, (b0, nb) in enumerate(TRIPLES):
                            rows = N * nb
                            xt = xs_pool.tile([120, TB, XC], bf16,
                                              tag=f"xs{k}", name=f"xs{k}")
                            for i in range(nb):
                                nc.sync.dma_start(
                                    out=xt[N * i:N * (i + 1), :, 0:F_IN],
                                    in_=Xnb[b0 + i, :, t:t + TB, :],
                                )
                            # merged one-hot: both categorical cols, all TB
                            # steps, all stacked graphs in one op
                            nc.vector.tensor_tensor(
                                out=xt[:rows, :, OH_P0:XC].rearrange(
                                    "p t (g k) -> p t g k", k=17
                                ),
                                in0=wt["iota2b"][0:rows, :].rearrange(
                                    "p (t g k) -> p t g k", t=TB, k=17
                                ),
                                in1=xt[
                                    :rows, :, NUM_CONT:NUM_CONT + 2
                                ].to_broadcast([rows, TB, 2, 17]),
                                op=EQ,
                            )
                            xraw[k] = xt

                    # ------- per-step transposes -> xfT [100, 320]
                    pxf = pxf_pool.tile([XC, R], bf16, tag="pxf")
                    for k, (b0, nb) in enumerate(TRIPLES):
                        rows = N * nb
                        nc.tensor.transpose(
                            pxf[:, 120 * k:120 * k + rows],
                            xraw[k][:rows, tl, :],
                            wt["identb"][:rows, :rows],
                        )
                    xfT = xf_pool.tile([XC, R], bf16, tag="xfT")
                    nc.any.tensor_copy(out=xfT[:], in_=pxf[:])

                    # ------- ts-1 broadcast across partitions via SWDGE DMA
                    # (gpsimd is otherwise idle; src re-reads one partition)
                    tsb = tsb_pool.tile([HID, R], f32, tag="tsb")
                    ts_row = tsm1_d[t % 64, t // 64, :]
                    nc.gpsimd.dma_start(
                        out=tsb[:],
                        in_=bass.AP(
                            tensor=ts_row.tensor,
                            offset=ts_row.offset,
                            ap=[[0, HID], [1, R]],
                        ),
                    )

                    # ------- c path: c_adj = c + tanh(Wd@c + bd) * (ts-1)
                    pd = pd_pool.tile([HID, R], f32, tag="pd")
                    nc.tensor.matmul(pd, wt["WdT"][:], cT[:], start=True, stop=True)
                    cs1 = work.tile([HID, R], f32, tag="cs1")
                    nc.scalar.activation(cs1[:], pd, AF.Tanh)
                    t1 = work.tile([HID, R], f32, tag="t1")
                    nc.vector.tensor_mul(t1[:], cs1[:], tsb[:])
                    cadj = work.tile([HID, R], f32, tag="cadj")
                    nc.vector.tensor_add(cadj[:], cT[0:HID, :].bitcast(f32), t1[:])

                    # ------- gates: psum[g] = WxT_g.T @ xfT + WallT_g.T @ h1
                    # split across two psum tiles (pgA double-buffered) so
                    # next step's x-side matmuls can start before sigmoid
                    # consumes the previous gates
                    pgA = pg_pool.tile([HID, 2, 512], f32, tag="pgA", bufs=2)
                    pgB = pg_pool.tile([HID, 2, 512], f32, tag="pgB", bufs=1)
                    halves = (pgA, pgB)
                    # order: (f,i) x then h parts first so sigmoid A can
                    # start while (o,ct) matmuls still run
                    for g in (0, 1):
                        nc.tensor.matmul(
                            halves[0][:, g, 0:R],
                            wt["WxT"][:, HID * g:HID * (g + 1)],
                            xfT[:], start=True, stop=False,
                        )
                    for g in (0, 1):
                        nc.tensor.matmul(
                            halves[0][:, g, 0:R],
                            wt["WallT"][:, HID * g:HID * (g + 1)],
                            hT[:], start=False, stop=True,
                        )
                    for g in (2, 3):
                        nc.tensor.matmul(
                            halves[1][:, g - 2, 0:R],
                            wt["WxT"][:, HID * g:HID * (g + 1)],
                            xfT[:], start=True, stop=False,
                        )
                    for g in (2, 3):
                        nc.tensor.matmul(
                            halves[1][:, g - 2, 0:R],
                            wt["WallT"][:, HID * g:HID * (g + 1)],
                            hT[:], start=False, stop=True,
                        )
                    gs = gsb_pool.tile([HID, 4, R], f32, tag="gs")
                    nc.scalar.activation(gs[:, 0:2, :], pgA[:, :, 0:R], AF.Sigmoid)
                    nc.scalar.activation(gs[:, 2:4, :], pgB[:, :, 0:R], AF.Sigmoid)

                    # ------- state update: c = f*c_adj + i*ct ; h = o*tanh(c)
                    t2 = work.tile([HID, R], f32, tag="t2")
                    nc.vector.tensor_mul(t2[:], gs[:, 0, :], cadj[:])
                    t3 = work.tile([HID, R], f32, tag="t3")
                    nc.vector.tensor_mul(t3[:], gs[:, 1, :], gs[:, 3, :])
                    nc.vector.tensor_add(cT[0:HID, :], t2[:], t3[:])
                    # tail split into column halves: tanh of half 0 overlaps
                    # the o*tanh multiply of half 1 (shortens the h chain)
                    tnc = work.tile([HID, R], f32, tag="tnc")
                    H2 = R // 2
                    for hh in range(2):
                        s0, s1 = H2 * hh, H2 * (hh + 1)
                        nc.scalar.activation(
                            tnc[:, s0:s1],
                            cT[0:HID, s0:s1].bitcast(f32),
                            AF.Tanh,
                        )
                        nc.vector.tensor_mul(
                            hT[0:HID, s0:s1], gs[:, 2, s0:s1], tnc[:, s0:s1]
                        )

                # ---- output linear: nodes = relu(lin @ h + lb)
                pl = pd_pool.tile([HID, R], f32, tag="pd")
                nc.tensor.matmul(pl, wt["linT"][:], hT[:], start=True, stop=True)
                nc.scalar.activation(nodesT[:], pl, AF.Relu)

            # ---------------- GCN: two SAGE layers + output proj
            with (
                tc.tile_pool(name="gc", bufs=2) as gc,
                tc.tile_pool(name="gcs", bufs=1) as gcs,
                tc.tile_pool(name="gp", bufs=2, space=PSUM) as gp,
                tc.tile_pool(name="gp1", bufs=1, space=PSUM) as gp1,
            ):
                def mean_agg(srcT, hid):
                    """srcT: [hid, R] feature-major -> aggT [hid, R]."""
                    aggT = gcs.tile([hid, R], f32r, tag=f"agg{hid}", name="aggT")
                    for b in range(BL):
                        cols = srcT[:, N * b:N * (b + 1)]   # [hid, 40] graph b
                        ptr = gp.tile([N, 128], f32, tag="ptr")
                        nc.tensor.transpose(
                            r(ptr[:, 0:hid]), cols, wt["ident"][:hid, :hid]
                        )
                        nbm = gc.tile([N, 128], f32r, tag="nbm")
                        nc.any.tensor_copy(out=nbm[:, 0:hid], in_=ptr[:, 0:hid])
                        pa = gp.tile([128, N], f32, tag="pa")
                        nc.tensor.matmul(
                            pa[0:hid, :], nbm[:, 0:hid], wt["Mmat"][:],
                            start=True, stop=True,
                        )
                        nc.any.tensor_copy(
                            out=aggT[:, N * b:N * (b + 1)], in_=pa[0:hid, :]
                        )
                    return aggT

                agg1 = mean_agg(nodesT, HID)
                pg1 = gp1.tile([64, R], f32, tag="pg1")
                nc.tensor.matmul(pg1, wt["s1l"][:], agg1[:], start=True, stop=False)
                nc.tensor.matmul(pg1, wt["s1r"][:], nodesT[:], start=False, stop=True)
                g1T = gcs.tile([64, R], f32r, tag="g1T")
                nc.scalar.activation(g1T[:], pg1, AF.Relu, bias=wt["s1b"][:].bitcast(f32))

                agg2 = mean_agg(g1T, 64)
                pg2 = gp1.tile([32, R], f32, tag="pg2")
                nc.tensor.matmul(pg2, wt["s2l"][:], agg2[:], start=True, stop=False)
                nc.tensor.matmul(pg2, wt["s2r"][:], g1T[:], start=False, stop=True)
                g2T = gcs.tile([32, R], f32r, tag="g2T")
                nc.scalar.activation(g2T[:], pg2, AF.Relu, bias=wt["s2b"][:].bitcast(f32))

                po = gp1.tile([2, R], f32, tag="po")
                nc.tensor.matmul(po, wt["ow"][:], g2T[:], start=True, stop=True)
                oT = gcs.tile([2, R], f32, tag="oT")
                nc.scalar.activation(oT[:], po, AF.Relu, bias=wt["ob"][:].bitcast(f32))

                nc.sync.dma_start(
                    out=out_ext.rearrange("k b n -> k (b n)"), in_=oT[:]
                )

    nc.compile()
    return nc


# ---------------------------------------------------------------- execution
_CACHE = {}


def _get_module():
    if "nc" not in _CACHE:
        _CACHE["nc"] = build_module()
    return _CACHE["nc"]


def make_in_maps(inputs):
    f32 = np.float32
    import ml_dtypes
    X = np.ascontiguousarray(np.asarray(inputs["X"], f32).astype(ml_dtypes.bfloat16))
    ts = np.ascontiguousarray(np.asarray(inputs["ts_list"], f32))
    wts = _host_weights(inputs)
    in_maps = []
    for c in range(NCORES):
        m = {"X": X[c * BL:(c + 1) * BL], "ts": ts[c * BL:(c + 1) * BL]}
        m.update(wts)
        in_maps.append(m)
    return in_maps


def kernel(**inputs) -> np.ndarray:
    from concourse.bass_utils import run_bass_kernel_spmd

    nc = _get_module()
    in_maps = make_in_maps(inputs)
    res = run_bass_kernel_spmd(nc, in_maps, list(range(NCORES)))
    outs = [
        np.transpose(res.results[c]["out"], (1, 2, 0)) for c in range(NCORES)
    ]
    return np.ascontiguousarray(np.concatenate(outs, axis=0).astype(np.float32))
